# revision 32
# baseline (speedup 1.0000x reference)
"""CSILoss (contrastive + rotation CE) Trainium2 kernel, v4.

Contract: kernel(**inputs) takes the FULL unsharded inputs
  z: [8192, 256] f32, rotation_predictions: [8192, 4] f32, labels: [8192] i64
and returns the full scalar loss (f32), computed on 8 NeuronCores.

Design (data-parallel over rows; each core owns a 1024-row slab):
- Host passes z TRANSPOSED in bf16, column-rotated per core so the own slab
  is always columns 0-1023 (position-independent SPMD program).
- All similarity tiles are computed in TRANSPOSED orientation [j, i] (sim is
  symmetric): 64 j-blocks of [128 j x 1024 i] via fp8 DoubleRow matmuls
  (lhsT = fp8(zT) block, rhs = fp8 normalized own-slab zsT8). Row sums over
  j are PARTITION sums = nearly-free PE ones-matmuls accumulating into one
  PSUM accumulator across all 64 blocks.
- Only the own slab is normalized explicitly; rnorm_j of the other operand
  folds into the per-partition exp scale (ACT scale AP / DVE scalar AP).
- exp split: ACT (native Exp) 40 blocks in a dedicated 2-deep PSUM ring of
  [128,1024] tiles; DVE (Schraudolph bf16 bit-pattern via one tensor_scalar)
  24 blocks in its own 2-deep ring of [128,512] tiles, so the engines never
  couple through buffer rotation. Pool does all bf16->fp8 conversion and
  some squaring, off the critical path.
- Column sums-of-squares from zT^2 (tensor_tensor, 2x mode) reduced by PE
  ones-matmuls; rsqrt via Quake seed + 1 Newton on DVE. All per-chunk state
  (za8/rnorm/ssq) lives in per-chunk tiles to avoid false tile-level deps.
- positives: pos_i = 2*|zn_i + zn_{i^1}|^2 - 4 (pair sums on Pool from the
  fp8 slab + PE reduction), computed at the tail; the masked diagonal is
  subtracted as the constant e^4 (error <= 1e-4 of S).
- final ln(S) via bit-trick log + one exp-based Newton refinement (the only
  ACT table ever loaded is the Exp set).
"""

import sys

for _p in ("/opt/trn_rl_repo", "/root/.axon_site/_ro/trn_rl_repo"):
    if _p not in sys.path:
        sys.path.insert(0, _p)

import math

import numpy as np

import concourse.bass as bass
import concourse.tile as tile
from concourse import bacc, mybir
from concourse.bass import ds, ts
from concourse.bass_utils import run_bass_kernel_spmd

B, D = 8192, 256
N_CORES = 8
SLAB = B // N_CORES  # 1024
NB = B // 128  # 64 j-blocks
NCH = 8  # chunks of 8 blocks
F32 = mybir.dt.float32
BF16 = mybir.dt.bfloat16
FP8 = mybir.dt.float8e4
I16 = mybir.dt.int16
I32 = mybir.dt.int32
U32 = mybir.dt.uint32
AF = mybir.ActivationFunctionType
ALU = mybir.AluOpType
DR = mybir.MatmulPerfMode.DoubleRow

LOG2E = 1.4426950408889634
SIG_E = 0.1167  # Schraudolph exp centering: tuned so the net loss bias ~0 at the 40A/24V split
A_SCH = 512.0 * LOG2E  # * rnorm_j -> per-partition scalar
B_SCH = 128.0 * (127.0 - SIG_E)
SIG_L = 0.0430
C1_LOG = math.log(2.0) / (1 << 23)
C2_LOG = -(127.0 - SIG_L) * math.log(2.0)
E4 = float(np.exp(4.0, dtype=np.float64))

# per-chunk engine pattern for exp blocks: 40 ACT / 24 DVE over 64 blocks
PATTERN = "AVAVAAVA"
PATTERN_LAST = "AVAAVAAA"  # 41 ACT / 23 DVE total
POOL_Z2T = set()  # Pool TT is too slow/late for in-band squaring

_CACHE = {}


def _build():
    nc = bacc.Bacc("TRN2", target_bir_lowering=False, debug=False)

    zt = nc.declare_dram_parameter("zt", [D, B], BF16, isOutput=False)
    zs = nc.declare_dram_parameter("zs", [SLAB, D], BF16, isOutput=False)
    rpoh = nc.declare_dram_parameter("rpoh", [128, 64], F32, isOutput=False)
    idmb = nc.declare_dram_parameter("idmb", [128, 128], BF16, isOutput=False)
    outv = nc.declare_dram_parameter("outv", [128, 24], F32, isOutput=True)

    with tile.TileContext(nc) as tc:
        from contextlib import ExitStack

        with ExitStack() as stk:
            const = stk.enter_context(tc.tile_pool(name="const", bufs=1))
            small = stk.enter_context(tc.tile_pool(name="small", bufs=1))
            ztp = stk.enter_context(tc.tile_pool(name="ztp", bufs=8))
            z2p = stk.enter_context(tc.tile_pool(name="z2p", bufs=2))
            ep = stk.enter_context(tc.tile_pool(name="ep", bufs=8))
            psA = stk.enter_context(tc.tile_pool(name="psA", bufs=2, space="PSUM"))
            psV = stk.enter_context(tc.tile_pool(name="psV", bufs=2, space="PSUM"))
            psS = stk.enter_context(tc.tile_pool(name="psS", bufs=1, space="PSUM"))
            psT = stk.enter_context(tc.tile_pool(name="psT", bufs=1, space="PSUM"))

            # ---------------- DMAs (SP sequencer, in order) ----------------
            ztb = [None] * NCH

            def dma_chunk(g):
                t = ztp.tile([128, 2, SLAB], BF16, tag="ztb", name=f"ztb{g}")
                nc.sync.dma_start(
                    out=t[:],
                    in_=zt[:, ds(SLAB * g, SLAB)].rearrange("(h p) j -> p h j", p=128),
                )
                ztb[g] = t

            t0 = ztp.tile([128, 2, SLAB], BF16, tag="ztb", name="ztb0")
            for hv in range(2):
                nc.sync.dma_start(
                    out=t0[:, :, ds(512 * hv, 512)],
                    in_=zt[:, ds(512 * hv, 512)].rearrange("(h p) j -> p h j", p=128),
                )
            ztb[0] = t0
            zs_sb = const.tile([128, 8, D], BF16, name="zs_sb")
            nc.sync.dma_start(
                out=zs_sb[:], in_=zs[:, :].rearrange("(b p) d -> p b d", p=128)
            )
            idmb_sb = const.tile([128, 128], BF16, name="idmb_sb")
            nc.sync.dma_start(out=idmb_sb[:], in_=idmb[:])
            dma_chunk(1)
            rpoh_sb = const.tile([128, 64], F32, name="rpoh_sb")
            nc.sync.dma_start(out=rpoh_sb[:], in_=rpoh[:])
            for g in range(2, NCH):
                dma_chunk(g)

            # ---------------- persistent SBUF ----------------
            za8 = [
                const.tile([128, 2, SLAB], FP8, name=f"za8_{g}") for g in range(NCH)
            ]
            zsT8 = const.tile([128, 2, SLAB], FP8, name="zsT8")
            rnorm4 = [
                small.tile([128, 8], F32, name=f"rnorm4_{g}") for g in range(NCH)
            ]
            rnormS = [
                small.tile([128, 8], F32, name=f"rnormS_{g}") for g in range(NCH)
            ]
            ssq = [small.tile([128, 8], F32, name=f"ssq_{g}") for g in range(NCH)]
            ones1 = const.tile([128, 1], BF16, name="ones1")
            nc.vector.memset(ones1[:], 1.0)
            ones1f = const.tile([128, 1], F32, name="ones1f")
            nc.vector.memset(ones1f[:], 1.0)

            ps_t = psT.tile([128, 512], F32, tag="pst", name="ps_t")

            def rsqrt_ops(dst_sl, src_sl, k, tag):
                # dst = rsqrt(src): Quake seed + 1 Newton (rel err ~2e-3)
                sbits = src_sl.bitcast(U32)
                hb = small.tile([128, k], I32, name=f"rs_h{tag}")
                nc.vector.tensor_scalar(
                    out=hb[:].bitcast(U32), in0=sbits, scalar1=1,
                    scalar2=None, op0=ALU.logical_shift_right,
                )
                sd = small.tile([128, k], I32, name=f"rs_s{tag}")
                nc.vector.tensor_scalar(
                    out=sd[:], in0=hb[:], scalar1=-1, scalar2=0x5F3759DF,
                    op0=ALU.mult, op1=ALU.add,
                )
                y = sd[:].bitcast(F32)
                y2 = small.tile([128, k], F32, name=f"rs_y2{tag}")
                hs = small.tile([128, k], F32, name=f"rs_hs{tag}")
                w = small.tile([128, k], F32, name=f"rs_w{tag}")
                nc.vector.tensor_scalar(
                    out=hs[:], in0=src_sl, scalar1=-0.5, scalar2=None, op0=ALU.mult
                )
                nc.vector.tensor_tensor(out=y2[:], in0=y, in1=y, op=ALU.mult)
                nc.vector.tensor_tensor(out=w[:], in0=y2[:], in1=hs[:], op=ALU.mult)
                nc.vector.tensor_scalar(
                    out=w[:], in0=w[:], scalar1=1.5, scalar2=None, op0=ALU.add
                )
                nc.vector.tensor_tensor(out=dst_sl, in0=y, in1=w[:], op=ALU.mult)

            def prep_sq(g):
                # zT^2 and column sums-of-squares via PE ones-matmuls
                z2 = z2p.tile([128, 2, SLAB], BF16, tag="z2", name=f"z2_{g}")
                eng = nc.gpsimd if g in POOL_Z2T else nc.vector
                eng.tensor_tensor(out=z2[:], in0=ztb[g][:], in1=ztb[g][:], op=ALU.mult)
                for b in range(8):
                    for h in range(2):
                        nc.tensor.matmul(
                            ps_t[:, 8 * g + b : 8 * g + b + 1],
                            lhsT=z2[:, h, ds(128 * b, 128)],
                            rhs=ones1[:],
                            start=(h == 0),
                            stop=(h == 1),
                        )
                nc.vector.tensor_copy(ssq[g][:], ps_t[:, ds(8 * g, 8)])

            def prep_rsqrt(g):
                r = small.tile([128, 8], F32, name=f"rsq{g}")
                rsqrt_ops(r[:, :], ssq[g][:], 8, f"c{g}")
                nc.vector.tensor_scalar(
                    out=rnorm4[g][:], in0=r[:], scalar1=4.0, scalar2=None, op0=ALU.mult
                )
                nc.vector.tensor_scalar(
                    out=rnormS[g][:], in0=r[:], scalar1=A_SCH, scalar2=None,
                    op0=ALU.mult,
                )
                return r

            def chunk_fp8(g):
                nc.gpsimd.tensor_copy(za8[g][:], ztb[g][:])

            # ---------------- lead-in (pipelined per half/row-tile) --------
            chunk_fp8(0)
            # squared chunk 0 in two halves so rsqrt can start early
            z2_0 = z2p.tile([128, 2, SLAB], BF16, tag="z2", name="z2_0")
            for hv in range(2):
                nc.vector.tensor_tensor(
                    out=z2_0[:, :, ds(512 * hv, 512)],
                    in0=ztb[0][:, :, ds(512 * hv, 512)],
                    in1=ztb[0][:, :, ds(512 * hv, 512)], op=ALU.mult,
                )
                for b in range(4 * hv, 4 * hv + 4):
                    for h in range(2):
                        nc.tensor.matmul(
                            ps_t[:, b : b + 1],
                            lhsT=z2_0[:, h, ds(128 * b, 128)],
                            rhs=ones1[:],
                            start=(h == 0),
                            stop=(h == 1),
                        )
            nc.vector.tensor_copy(ssq[0][:], ps_t[:, 0:8])
            rsq0h = [small.tile([128, 4], F32, name=f"rsq0h{hv}") for hv in range(2)]
            zn_slab = const.tile([128, 8, D], BF16, name="zn_slab")
            ps_sl = [
                psA.tile([128, SLAB], F32, tag="psa", name=f"ps_slab{h}")
                for h in range(2)
            ]
            for hv in range(2):
                rsqrt_ops(rsq0h[hv][:, :], ssq[0][:, ds(4 * hv, 4)], 4, f"c0h{hv}")
                for b in range(4 * hv, 4 * hv + 4):
                    nc.vector.tensor_scalar(
                        out=zn_slab[:, b, :], in0=zs_sb[:, b, :],
                        scalar1=rsq0h[hv][:, b - 4 * hv : b - 4 * hv + 1],
                        scalar2=None, op0=ALU.mult,
                    )
                    for h in range(2):
                        nc.tensor.matmul(
                            ps_sl[h][:, ds(128 * b, 128)],
                            lhsT=zn_slab[:, b, ds(128 * h, 128)],
                            rhs=idmb_sb[:],
                            start=True,
                            stop=True,
                        )
            nc.vector.tensor_copy(zsT8[:, 0, :], ps_sl[0][:])
            nc.scalar.copy(zsT8[:, 1, :], ps_sl[1][:])
            # rnorm scales for chunk 0 (from the two rsqrt halves)
            for hv in range(2):
                nc.vector.tensor_scalar(
                    out=rnorm4[0][:, ds(4 * hv, 4)], in0=rsq0h[hv][:], scalar1=4.0,
                    scalar2=None, op0=ALU.mult,
                )
                nc.vector.tensor_scalar(
                    out=rnormS[0][:, ds(4 * hv, 4)], in0=rsq0h[hv][:], scalar1=A_SCH,
                    scalar2=None, op0=ALU.mult,
                )

            # rotation exps (loads the Exp table while ACT is otherwise idle)
            rexp = small.tile([128, 8, 4], F32, name="rexp")
            nc.scalar.activation(
                out=rexp[:],
                in_=rpoh_sb[:, 0:32].rearrange("p (b f) -> p b f", f=4),
                func=AF.Exp,
            )

            # ---------------- main loop ----------------
            S_acc = psS.tile([128, 512], F32, tag="pss", name="S_acc")
            nmm = [0]
            pend = []

            def emit_rowsums(e):
                for k in range(8):
                    nc.tensor.matmul(
                        S_acc[:, k : k + 1],
                        lhsT=e[:, ds(128 * k, 128)],
                        rhs=ones1[:],
                        start=(nmm[0] == 0),
                        stop=(nmm[0] == 8 * NB - 1),
                    )
                    nmm[0] += 1

            def exp_block(g, b, eng):
                t = 8 * g + b
                e = ep.tile([128, SLAB], BF16, tag="e", name=f"e{t}")
                if eng == "A":
                    ps = psA.tile([128, SLAB], F32, tag="psa", name=f"psl{t}")
                    for s in range(2):
                        nc.tensor.matmul(
                            ps[:, ds(512 * s, 512)],
                            lhsT=za8[g][:, :, ds(128 * b, 128)],
                            rhs=zsT8[:, :, ds(512 * s, 512)],
                            start=True,
                            stop=True,
                            perf_mode=DR,
                        )
                    nc.scalar.activation(
                        out=e[:], in_=ps[:], func=AF.Exp, scale=rnorm4[g][:, b : b + 1]
                    )
                else:
                    for s in range(2):
                        ps = psV.tile([128, 512], F32, tag="psv", name=f"psl{t}_{s}")
                        nc.tensor.matmul(
                            ps[:],
                            lhsT=za8[g][:, :, ds(128 * b, 128)],
                            rhs=zsT8[:, :, ds(512 * s, 512)],
                            start=True,
                            stop=True,
                            perf_mode=DR,
                        )
                        nc.vector.tensor_scalar(
                            out=e[:, ds(512 * s, 512)].bitcast(I16), in0=ps[:],
                            scalar1=rnormS[g][:, b : b + 1], scalar2=B_SCH,
                            op0=ALU.mult, op1=ALU.add,
                        )
                pend.append(e)
                if len(pend) > 4:
                    emit_rowsums(pend.pop(0))

            for g in range(NCH):
                pat = PATTERN
                for b in range(8):
                    exp_block(g, b, pat[b])
                    if g + 1 < NCH:
                        if b == 1:
                            chunk_fp8(g + 1)
                        elif b == 3:
                            prep_sq(g + 1)
                        elif b == 5:
                            prep_rsqrt(g + 1)
            while pend:
                emit_rowsums(pend.pop(0))

            # ---------------- positives (tail; feeds only the finals) ------
            vpair = const.tile([128, 2, 512], BF16, name="vpair")
            nc.gpsimd.tensor_tensor(
                out=vpair[:], in0=zsT8[:, :, 0 : SLAB : 2],
                in1=zsT8[:, :, 1 : SLAB : 2], op=ALU.add,
            )
            w2 = const.tile([128, 2, 512], BF16, name="w2")
            nc.gpsimd.tensor_tensor(out=w2[:], in0=vpair[:], in1=vpair[:], op=ALU.mult)
            for k in range(4):
                for h in range(2):
                    nc.tensor.matmul(
                        ps_t[:, 96 + k : 97 + k],
                        lhsT=w2[:, h, ds(128 * k, 128)],
                        rhs=ones1[:],
                        start=(h == 0),
                        stop=(h == 1),
                    )
            posw = small.tile([128, 4], F32, name="posw")
            nc.vector.tensor_copy(posw[:], ps_t[:, 96:100])

            # ---------------- finals ----------------
            # logv = [S - e^4 | rot sums]; lse via bit-log + Newton refine
            logv = small.tile([128, 16], F32, name="logv")
            nc.vector.tensor_scalar(
                out=logv[:, 0:8], in0=S_acc[:, 0:8], scalar1=1.0, scalar2=-E4,
                op0=ALU.mult, op1=ALU.add,
            )
            nc.vector.reduce_sum(logv[:, 8:16], rexp[:], axis=mybir.AxisListType.X)

            bits_f = small.tile([128, 16], F32, name="bits_f")
            nc.vector.tensor_copy(bits_f[:], logv[:].bitcast(I32))
            y0 = small.tile([128, 16], F32, name="y0")
            nc.vector.tensor_scalar(
                out=y0[:], in0=bits_f[:], scalar1=C1_LOG, scalar2=C2_LOG,
                op0=ALU.mult, op1=ALU.add,
            )
            en = small.tile([128, 16], F32, name="en")
            nc.scalar.activation(out=en[:], in_=y0[:], func=AF.Exp, scale=-1.0)
            r_ = small.tile([128, 16], F32, name="r_")
            nc.vector.tensor_tensor(out=r_[:], in0=logv[:], in1=en[:], op=ALU.mult)

            # host finishes the reduction: out = [lse(16) | posw(4) | picked | pad]
            ov = small.tile([128, 24], F32, name="ov")
            nc.vector.scalar_tensor_tensor(
                out=ov[:, 0:16], in0=y0[:], scalar=-1.0, in1=r_[:],
                op0=ALU.add, op1=ALU.add,
            )
            pscr = small.tile([128, 32], F32, name="pscr")
            nc.vector.scalar_tensor_tensor(
                out=pscr[:], in0=rpoh_sb[:, 0:32], scalar=1.0, in1=rpoh_sb[:, 32:64],
                op0=ALU.mult, op1=ALU.mult, accum_out=ov[:, 20:21],
            )
            nc.vector.tensor_copy(ov[:, 16:20], posw[:])
            nc.vector.memset(ov[:, 21:24], 0.0)
            nc.sync.dma_start(out=outv[:], in_=ov[:])

    nc.compile()
    return nc


def get_nc():
    if "nc" not in _CACHE:
        _CACHE["nc"] = _build()
    return _CACHE["nc"]


def _host_inputs(z, rotation_predictions, labels):
    import ml_dtypes

    z = np.ascontiguousarray(np.asarray(z, dtype=np.float32))
    rp = np.ascontiguousarray(np.asarray(rotation_predictions, dtype=np.float32))
    lab = np.asarray(labels).astype(np.int64)
    oh_full = np.eye(4, dtype=np.float32)[lab % 4]

    zb = z.astype(ml_dtypes.bfloat16)
    zt0 = np.ascontiguousarray(zb.T)  # [256, 8192] bf16
    idmb = np.eye(128, dtype=ml_dtypes.bfloat16)

    in_maps = []
    for c in range(N_CORES):
        r0, r1 = c * SLAB, (c + 1) * SLAB
        rp_s = rp[r0:r1].reshape(8, 128, 4).transpose(1, 0, 2).reshape(128, 32)
        oh_s = oh_full[r0:r1].reshape(8, 128, 4).transpose(1, 0, 2).reshape(128, 32)
        rpoh = np.ascontiguousarray(np.concatenate([rp_s, oh_s], axis=1))
        in_maps.append(
            {
                "zt": np.ascontiguousarray(np.roll(zt0, -SLAB * c, axis=1)),
                "zs": np.ascontiguousarray(zb[r0:r1]),
                "rpoh": rpoh,
                "idmb": idmb,
            }
        )
    return in_maps


def kernel(z, rotation_predictions, labels):
    nc = get_nc()
    in_maps = _host_inputs(z, rotation_predictions, labels)
    res = run_bass_kernel_spmd(nc, in_maps, core_ids=list(range(N_CORES)))
    total = 0.0
    for c in range(N_CORES):
        ov = np.asarray(res.results[c]["outv"], dtype=np.float64)
        lse_sum = ov[:, 0:16].sum()
        pos_sum = 4.0 * ov[:, 16:20].sum() - 32.0 * 128
        picked_sum = ov[:, 20].sum()
        total += lse_sum - pos_sum - picked_sum
    return np.float32(total / B)


if __name__ == "__main__":
    rng = np.random.default_rng(0)
    z = rng.standard_normal((B, D), dtype=np.float32)
    rp = rng.standard_normal((B, 4), dtype=np.float32)
    lab = rng.integers(0, 4, size=(B,)).astype(np.int64)
    print("loss:", kernel(z, rp, lab))


# revision 40
# speedup vs baseline: 1.0202x; 1.0202x over previous
"""CSILoss (contrastive + rotation CE) Trainium2 kernel, v4.

Contract: kernel(**inputs) takes the FULL unsharded inputs
  z: [8192, 256] f32, rotation_predictions: [8192, 4] f32, labels: [8192] i64
and returns the full scalar loss (f32), computed on 8 NeuronCores.

Design (data-parallel over rows; each core owns a 1024-row slab):
- Host passes z TRANSPOSED in bf16, column-rotated per core so the own slab
  is always columns 0-1023 (position-independent SPMD program).
- All similarity tiles are computed in TRANSPOSED orientation [j, i] (sim is
  symmetric): 64 j-blocks of [128 j x 1024 i] via fp8 DoubleRow matmuls
  (lhsT = fp8(zT) block, rhs = fp8 normalized own-slab zsT8). Row sums over
  j are PARTITION sums = nearly-free PE ones-matmuls accumulating into one
  PSUM accumulator across all 64 blocks.
- Only the own slab is normalized explicitly; rnorm_j of the other operand
  folds into the per-partition exp scale (ACT scale AP / DVE scalar AP).
- exp split: ACT (native Exp) 40 blocks in a dedicated 2-deep PSUM ring of
  [128,1024] tiles; DVE (Schraudolph bf16 bit-pattern via one tensor_scalar)
  24 blocks in its own 2-deep ring of [128,512] tiles, so the engines never
  couple through buffer rotation. Pool does all bf16->fp8 conversion and
  some squaring, off the critical path.
- Column sums-of-squares from zT^2 (tensor_tensor, 2x mode) reduced by PE
  ones-matmuls; rsqrt via Quake seed + 1 Newton on DVE. All per-chunk state
  (za8/rnorm/ssq) lives in per-chunk tiles to avoid false tile-level deps.
- positives: pos_i = 2*|zn_i + zn_{i^1}|^2 - 4 (pair sums on Pool from the
  fp8 slab + PE reduction), computed at the tail; the masked diagonal is
  subtracted as the constant e^4 (error <= 1e-4 of S).
- final ln(S) via bit-trick log + one exp-based Newton refinement (the only
  ACT table ever loaded is the Exp set).
"""

import sys

for _p in ("/opt/trn_rl_repo", "/root/.axon_site/_ro/trn_rl_repo"):
    if _p not in sys.path:
        sys.path.insert(0, _p)

import math

import numpy as np

import concourse.bass as bass
import concourse.tile as tile
from concourse import bacc, mybir
from concourse.bass import ds, ts
from concourse.bass_utils import run_bass_kernel_spmd

B, D = 8192, 256
N_CORES = 8
SLAB = B // N_CORES  # 1024
NB = B // 128  # 64 j-blocks
NCH = 8  # chunks of 8 blocks
F32 = mybir.dt.float32
BF16 = mybir.dt.bfloat16
FP8 = mybir.dt.float8e4
I16 = mybir.dt.int16
I32 = mybir.dt.int32
U32 = mybir.dt.uint32
AF = mybir.ActivationFunctionType
ALU = mybir.AluOpType
DR = mybir.MatmulPerfMode.DoubleRow

LOG2E = 1.4426950408889634
SIG_E = 0.1194  # Schraudolph exp centering: tuned so the net loss bias ~0 at the 41A/23V split
A_SCH = 512.0 * LOG2E  # * rnorm_j -> per-partition scalar
B_SCH = 128.0 * (127.0 - SIG_E)
SIG_L = 0.0430
C1_LOG = math.log(2.0) / (1 << 23)
C2_LOG = -(127.0 - SIG_L) * math.log(2.0)
E4 = float(np.exp(4.0, dtype=np.float64))

# per-chunk engine pattern for exp blocks: 40 ACT / 24 DVE over 64 blocks
PATTERN = "AVAVAAVA"
PATTERN_LAST = "AVAAVAAA"  # 41 ACT / 23 DVE total
POOL_Z2T = set()  # Pool TT is too slow/late for in-band squaring

_CACHE = {}


def _build():
    nc = bacc.Bacc("TRN2", target_bir_lowering=False, debug=False)

    zt = nc.declare_dram_parameter("zt", [D, B], BF16, isOutput=False)
    zs = nc.declare_dram_parameter("zs", [SLAB, D], BF16, isOutput=False)
    rpoh = nc.declare_dram_parameter("rpoh", [128, 64], F32, isOutput=False)
    idmb = nc.declare_dram_parameter("idmb", [128, 128], BF16, isOutput=False)
    outv = nc.declare_dram_parameter("outv", [128, 24], F32, isOutput=True)

    with tile.TileContext(nc) as tc:
        from contextlib import ExitStack

        with ExitStack() as stk:
            const = stk.enter_context(tc.tile_pool(name="const", bufs=1))
            small = stk.enter_context(tc.tile_pool(name="small", bufs=1))
            ztp = stk.enter_context(tc.tile_pool(name="ztp", bufs=8))
            z2p = stk.enter_context(tc.tile_pool(name="z2p", bufs=2))
            ep = stk.enter_context(tc.tile_pool(name="ep", bufs=8))
            psA = stk.enter_context(tc.tile_pool(name="psA", bufs=2, space="PSUM"))
            psV = stk.enter_context(tc.tile_pool(name="psV", bufs=2, space="PSUM"))
            psS = stk.enter_context(tc.tile_pool(name="psS", bufs=1, space="PSUM"))
            psT = stk.enter_context(tc.tile_pool(name="psT", bufs=1, space="PSUM"))

            # ---------------- DMAs (SP sequencer, in order) ----------------
            ztb = [None] * NCH

            def dma_chunk(g):
                t = ztp.tile([128, 2, SLAB], BF16, tag="ztb", name=f"ztb{g}")
                nc.sync.dma_start(
                    out=t[:],
                    in_=zt[:, ds(SLAB * g, SLAB)].rearrange("(h p) j -> p h j", p=128),
                )
                ztb[g] = t

            t0 = ztp.tile([128, 2, SLAB], BF16, tag="ztb", name="ztb0")
            for hv in range(2):
                nc.sync.dma_start(
                    out=t0[:, :, ds(512 * hv, 512)],
                    in_=zt[:, ds(512 * hv, 512)].rearrange("(h p) j -> p h j", p=128),
                )
            ztb[0] = t0
            zs_sb = const.tile([128, 8, D], BF16, name="zs_sb")
            nc.sync.dma_start(
                out=zs_sb[:], in_=zs[:, :].rearrange("(b p) d -> p b d", p=128)
            )
            idmb_sb = const.tile([128, 128], BF16, name="idmb_sb")
            nc.sync.dma_start(out=idmb_sb[:], in_=idmb[:])
            dma_chunk(1)
            rpoh_sb = const.tile([128, 64], F32, name="rpoh_sb")
            nc.sync.dma_start(out=rpoh_sb[:], in_=rpoh[:])
            for g in range(2, NCH):
                dma_chunk(g)

            # ---------------- persistent SBUF ----------------
            za8 = [
                const.tile([128, 2, SLAB], FP8, name=f"za8_{g}") for g in range(NCH)
            ]
            zsT8 = const.tile([128, 2, SLAB], FP8, name="zsT8")
            rnorm4 = [
                small.tile([128, 8], F32, name=f"rnorm4_{g}") for g in range(NCH)
            ]
            rnormS = [
                small.tile([128, 8], F32, name=f"rnormS_{g}") for g in range(NCH)
            ]
            ssq = [small.tile([128, 8], F32, name=f"ssq_{g}") for g in range(NCH)]
            ones1 = const.tile([128, 1], BF16, name="ones1")
            nc.vector.memset(ones1[:], 1.0)
            ones1f = const.tile([128, 1], F32, name="ones1f")
            nc.vector.memset(ones1f[:], 1.0)

            ps_t = psT.tile([128, 512], F32, tag="pst", name="ps_t")

            def rsqrt_ops(dst_sl, src_sl, k, tag):
                # dst = rsqrt(src): Quake seed + 1 Newton (rel err ~2e-3)
                sbits = src_sl.bitcast(U32)
                hb = small.tile([128, k], I32, name=f"rs_h{tag}")
                nc.vector.tensor_scalar(
                    out=hb[:].bitcast(U32), in0=sbits, scalar1=1,
                    scalar2=None, op0=ALU.logical_shift_right,
                )
                sd = small.tile([128, k], I32, name=f"rs_s{tag}")
                nc.vector.tensor_scalar(
                    out=sd[:], in0=hb[:], scalar1=-1, scalar2=0x5F3759DF,
                    op0=ALU.mult, op1=ALU.add,
                )
                y = sd[:].bitcast(F32)
                y2 = small.tile([128, k], F32, name=f"rs_y2{tag}")
                hs = small.tile([128, k], F32, name=f"rs_hs{tag}")
                w = small.tile([128, k], F32, name=f"rs_w{tag}")
                nc.vector.tensor_scalar(
                    out=hs[:], in0=src_sl, scalar1=-0.5, scalar2=None, op0=ALU.mult
                )
                nc.vector.tensor_tensor(out=y2[:], in0=y, in1=y, op=ALU.mult)
                nc.vector.tensor_tensor(out=w[:], in0=y2[:], in1=hs[:], op=ALU.mult)
                nc.vector.tensor_scalar(
                    out=w[:], in0=w[:], scalar1=1.5, scalar2=None, op0=ALU.add
                )
                nc.vector.tensor_tensor(out=dst_sl, in0=y, in1=w[:], op=ALU.mult)

            def prep_sq(g):
                # zT^2 and column sums-of-squares via PE ones-matmuls
                z2 = z2p.tile([128, 2, SLAB], BF16, tag="z2", name=f"z2_{g}")
                eng = nc.gpsimd if g in POOL_Z2T else nc.vector
                eng.tensor_tensor(out=z2[:], in0=ztb[g][:], in1=ztb[g][:], op=ALU.mult)
                for b in range(8):
                    for h in range(2):
                        nc.tensor.matmul(
                            ps_t[:, 8 * g + b : 8 * g + b + 1],
                            lhsT=z2[:, h, ds(128 * b, 128)],
                            rhs=ones1[:],
                            start=(h == 0),
                            stop=(h == 1),
                        )
                nc.vector.tensor_copy(ssq[g][:], ps_t[:, ds(8 * g, 8)])

            def prep_rsqrt(g):
                r = small.tile([128, 8], F32, name=f"rsq{g}")
                rsqrt_ops(r[:, :], ssq[g][:], 8, f"c{g}")
                nc.vector.tensor_scalar(
                    out=rnorm4[g][:], in0=r[:], scalar1=4.0, scalar2=None, op0=ALU.mult
                )
                nc.vector.tensor_scalar(
                    out=rnormS[g][:], in0=r[:], scalar1=A_SCH, scalar2=None,
                    op0=ALU.mult,
                )
                return r

            def chunk_fp8(g):
                nc.gpsimd.tensor_copy(za8[g][:], ztb[g][:])

            # ---------------- lead-in (pipelined per half/row-tile) --------
            chunk_fp8(0)
            # squared chunk 0 in two halves so rsqrt can start early
            z2_0 = z2p.tile([128, 2, SLAB], BF16, tag="z2", name="z2_0")
            for hv in range(2):
                nc.vector.tensor_tensor(
                    out=z2_0[:, :, ds(512 * hv, 512)],
                    in0=ztb[0][:, :, ds(512 * hv, 512)],
                    in1=ztb[0][:, :, ds(512 * hv, 512)], op=ALU.mult,
                )
                for b in range(4 * hv, 4 * hv + 4):
                    for h in range(2):
                        nc.tensor.matmul(
                            ps_t[:, b : b + 1],
                            lhsT=z2_0[:, h, ds(128 * b, 128)],
                            rhs=ones1[:],
                            start=(h == 0),
                            stop=(h == 1),
                        )
            nc.vector.tensor_copy(ssq[0][:], ps_t[:, 0:8])
            rsq0h = [small.tile([128, 4], F32, name=f"rsq0h{hv}") for hv in range(2)]
            zn_slab = const.tile([128, 8, D], BF16, name="zn_slab")
            ps_sl = [
                psA.tile([128, SLAB], F32, tag="psa", name=f"ps_slab{h}")
                for h in range(2)
            ]
            for hv in range(2):
                rsqrt_ops(rsq0h[hv][:, :], ssq[0][:, ds(4 * hv, 4)], 4, f"c0h{hv}")
                for b in range(4 * hv, 4 * hv + 4):
                    nc.vector.tensor_scalar(
                        out=zn_slab[:, b, :], in0=zs_sb[:, b, :],
                        scalar1=rsq0h[hv][:, b - 4 * hv : b - 4 * hv + 1],
                        scalar2=None, op0=ALU.mult,
                    )
                    for h in range(2):
                        nc.tensor.matmul(
                            ps_sl[h][:, ds(128 * b, 128)],
                            lhsT=zn_slab[:, b, ds(128 * h, 128)],
                            rhs=idmb_sb[:],
                            start=True,
                            stop=True,
                        )
            nc.vector.tensor_copy(zsT8[:, 0, :], ps_sl[0][:])
            nc.scalar.copy(zsT8[:, 1, :], ps_sl[1][:])
            # rnorm scales for chunk 0 (from the two rsqrt halves)
            for hv in range(2):
                nc.vector.tensor_scalar(
                    out=rnorm4[0][:, ds(4 * hv, 4)], in0=rsq0h[hv][:], scalar1=4.0,
                    scalar2=None, op0=ALU.mult,
                )
                nc.vector.tensor_scalar(
                    out=rnormS[0][:, ds(4 * hv, 4)], in0=rsq0h[hv][:], scalar1=A_SCH,
                    scalar2=None, op0=ALU.mult,
                )

            # rotation exps (loads the Exp table while ACT is otherwise idle)
            rexp = small.tile([128, 8, 4], F32, name="rexp")
            nc.scalar.activation(
                out=rexp[:],
                in_=rpoh_sb[:, 0:32].rearrange("p (b f) -> p b f", f=4),
                func=AF.Exp,
            )

            # ---------------- main loop ----------------
            S_acc = psS.tile([128, 512], F32, tag="pss", name="S_acc")
            nmm = [0]
            pend = []

            def emit_rowsums(e):
                for k in range(8):
                    nc.tensor.matmul(
                        S_acc[:, k : k + 1],
                        lhsT=e[:, ds(128 * k, 128)],
                        rhs=ones1[:],
                        start=(nmm[0] == 0),
                        stop=(nmm[0] == 8 * NB - 1),
                    )
                    nmm[0] += 1

            def exp_block(g, b, eng):
                t = 8 * g + b
                e = ep.tile([128, SLAB], BF16, tag="e", name=f"e{t}")
                if eng == "A":
                    ps = psA.tile([128, SLAB], F32, tag="psa", name=f"psl{t}")
                    for s in range(2):
                        nc.tensor.matmul(
                            ps[:, ds(512 * s, 512)],
                            lhsT=za8[g][:, :, ds(128 * b, 128)],
                            rhs=zsT8[:, :, ds(512 * s, 512)],
                            start=True,
                            stop=True,
                            perf_mode=DR,
                        )
                    nc.scalar.activation(
                        out=e[:], in_=ps[:], func=AF.Exp, scale=rnorm4[g][:, b : b + 1]
                    )
                else:
                    for s in range(2):
                        ps = psV.tile([128, 512], F32, tag="psv", name=f"psl{t}_{s}")
                        nc.tensor.matmul(
                            ps[:],
                            lhsT=za8[g][:, :, ds(128 * b, 128)],
                            rhs=zsT8[:, :, ds(512 * s, 512)],
                            start=True,
                            stop=True,
                            perf_mode=DR,
                        )
                        nc.vector.tensor_scalar(
                            out=e[:, ds(512 * s, 512)].bitcast(I16), in0=ps[:],
                            scalar1=rnormS[g][:, b : b + 1], scalar2=B_SCH,
                            op0=ALU.mult, op1=ALU.add,
                        )
                pend.append(e)
                if len(pend) > 4:
                    emit_rowsums(pend.pop(0))

            for g in range(NCH):
                pat = "AVAVAAAA" if g == 3 else PATTERN
                for b in range(8):
                    exp_block(g, b, pat[b])
                    if g + 1 < NCH:
                        if b == 1:
                            chunk_fp8(g + 1)
                        elif b == 3:
                            prep_sq(g + 1)
                        elif b == 5:
                            prep_rsqrt(g + 1)
            while pend:
                emit_rowsums(pend.pop(0))

            # ---------------- positives (tail; feeds only the finals) ------
            vpair = const.tile([128, 2, 512], BF16, name="vpair")
            nc.gpsimd.tensor_tensor(
                out=vpair[:], in0=zsT8[:, :, 0 : SLAB : 2],
                in1=zsT8[:, :, 1 : SLAB : 2], op=ALU.add,
            )
            w2 = const.tile([128, 2, 512], BF16, name="w2")
            nc.gpsimd.tensor_tensor(out=w2[:], in0=vpair[:], in1=vpair[:], op=ALU.mult)
            for k in range(4):
                for h in range(2):
                    nc.tensor.matmul(
                        ps_t[:, 96 + k : 97 + k],
                        lhsT=w2[:, h, ds(128 * k, 128)],
                        rhs=ones1[:],
                        start=(h == 0),
                        stop=(h == 1),
                    )
            posw = small.tile([128, 4], F32, name="posw")
            nc.vector.tensor_copy(posw[:], ps_t[:, 96:100])

            # ---------------- finals ----------------
            # logv = [S - e^4 | rot sums]; lse via bit-log + Newton refine
            logv = small.tile([128, 16], F32, name="logv")
            nc.vector.tensor_scalar(
                out=logv[:, 0:8], in0=S_acc[:, 0:8], scalar1=1.0, scalar2=-E4,
                op0=ALU.mult, op1=ALU.add,
            )
            nc.vector.reduce_sum(logv[:, 8:16], rexp[:], axis=mybir.AxisListType.X)

            bits_f = small.tile([128, 16], F32, name="bits_f")
            nc.vector.tensor_copy(bits_f[:], logv[:].bitcast(I32))
            y0 = small.tile([128, 16], F32, name="y0")
            nc.vector.tensor_scalar(
                out=y0[:], in0=bits_f[:], scalar1=C1_LOG, scalar2=C2_LOG,
                op0=ALU.mult, op1=ALU.add,
            )
            en = small.tile([128, 16], F32, name="en")
            nc.scalar.activation(out=en[:], in_=y0[:], func=AF.Exp, scale=-1.0)
            r_ = small.tile([128, 16], F32, name="r_")
            nc.vector.tensor_tensor(out=r_[:], in0=logv[:], in1=en[:], op=ALU.mult)

            # host finishes the reduction: out = [lse(16) | posw(4) | picked | pad]
            ov = small.tile([128, 24], F32, name="ov")
            nc.vector.scalar_tensor_tensor(
                out=ov[:, 0:16], in0=y0[:], scalar=-1.0, in1=r_[:],
                op0=ALU.add, op1=ALU.add,
            )
            pscr = small.tile([128, 32], F32, name="pscr")
            nc.vector.scalar_tensor_tensor(
                out=pscr[:], in0=rpoh_sb[:, 0:32], scalar=1.0, in1=rpoh_sb[:, 32:64],
                op0=ALU.mult, op1=ALU.mult, accum_out=ov[:, 20:21],
            )
            nc.vector.tensor_copy(ov[:, 16:20], posw[:])
            nc.vector.memset(ov[:, 21:24], 0.0)
            nc.sync.dma_start(out=outv[:], in_=ov[:])

    nc.compile()
    return nc


def get_nc():
    if "nc" not in _CACHE:
        _CACHE["nc"] = _build()
    return _CACHE["nc"]


def _host_inputs(z, rotation_predictions, labels):
    import ml_dtypes

    z = np.ascontiguousarray(np.asarray(z, dtype=np.float32))
    rp = np.ascontiguousarray(np.asarray(rotation_predictions, dtype=np.float32))
    lab = np.asarray(labels).astype(np.int64)
    oh_full = np.eye(4, dtype=np.float32)[lab % 4]

    zb = z.astype(ml_dtypes.bfloat16)
    zt0 = np.ascontiguousarray(zb.T)  # [256, 8192] bf16
    idmb = np.eye(128, dtype=ml_dtypes.bfloat16)

    in_maps = []
    for c in range(N_CORES):
        r0, r1 = c * SLAB, (c + 1) * SLAB
        rp_s = rp[r0:r1].reshape(8, 128, 4).transpose(1, 0, 2).reshape(128, 32)
        oh_s = oh_full[r0:r1].reshape(8, 128, 4).transpose(1, 0, 2).reshape(128, 32)
        rpoh = np.ascontiguousarray(np.concatenate([rp_s, oh_s], axis=1))
        in_maps.append(
            {
                "zt": np.ascontiguousarray(np.roll(zt0, -SLAB * c, axis=1)),
                "zs": np.ascontiguousarray(zb[r0:r1]),
                "rpoh": rpoh,
                "idmb": idmb,
            }
        )
    return in_maps


def kernel(z, rotation_predictions, labels):
    nc = get_nc()
    in_maps = _host_inputs(z, rotation_predictions, labels)
    res = run_bass_kernel_spmd(nc, in_maps, core_ids=list(range(N_CORES)))
    total = 0.0
    for c in range(N_CORES):
        ov = np.asarray(res.results[c]["outv"], dtype=np.float64)
        lse_sum = ov[:, 0:16].sum()
        pos_sum = 4.0 * ov[:, 16:20].sum() - 32.0 * 128
        picked_sum = ov[:, 20].sum()
        total += lse_sum - pos_sum - picked_sum
    return np.float32(total / B)


if __name__ == "__main__":
    rng = np.random.default_rng(0)
    z = rng.standard_normal((B, D), dtype=np.float32)
    rp = rng.standard_normal((B, 4), dtype=np.float32)
    lab = rng.integers(0, 4, size=(B,)).astype(np.int64)
    print("loss:", kernel(z, rp, lab))
